# revision 17
# baseline (speedup 1.0000x reference)
"""Trainium2 Bass kernel for DTPTrajRec-style scatter + embedding gathers.

Computes, for full inputs:
  - new_grid/new_gps scatter (last-write-wins per (trajectory, time slot))
  - src_emb = E1[new_grid[...,0]] ++ E2[new_grid[...,1]]   [B, T, 2D]
  - pre_emb/next_emb = E1[g[...,0]] ++ E2[g[...,1]]        [T, B, 2D]
Sharded over 8 NeuronCores: batch-parallel for the src/scatter path,
T-parallel for pre/next. Embedding tables are replicated (concatenated
into one [1026, 512] table so one indirect-DMA gather fetches either).

Last-write-wins winner (max source index j per (trajectory, slot)) is
computed on the vector engine: 16 fused is_equal*(j+1) compares against
a slot-iota, max-accumulated, then partition-folded. Source points are
interleaved on the host (partition p holds points of trajectory p%8) so
partition folds stay within one trajectory.
"""
import sys

for _p in ("/opt/trn_rl_repo", "/root/.axon_site/_ro/trn_rl_repo"):
    if _p not in sys.path:
        sys.path.insert(0, _p)

from contextlib import ExitStack

import numpy as np

import concourse.bass as bass
import concourse.tile as tile
from concourse import bacc, mybir
from concourse.bass_utils import run_bass_kernel_spmd

P = 128
NCORES = 8
B, L, T = 64, 256, 512
D = 512
VC = 2 * 513            # combined table rows (E1 then E2)
BC = B // NCORES        # 8 trajectories per core
TC = T // NCORES        # 64 pre/next time rows per core
NPTS = BC * L           # 2048 source points per core
NSLOT = BC * T          # 4096 output rows per core (= TC * B as well)
PPP = NPTS // P         # 16 points per partition
SPP = NSLOT // P        # 32 output rows per partition
KG = 8                  # embedding rows gathered per indirect DMA per partition
NG = SPP // KG          # 4 gather groups per output tensor

F32 = mybir.dt.float32
I32 = mybir.dt.int32
ADD = mybir.AluOpType.add
SUB = mybir.AluOpType.subtract
MULT = mybir.AluOpType.mult
MAX = mybir.AluOpType.max


def build_nc():
    nc = bacc.Bacc("TRN2", target_bir_lowering=False, debug=False,
                   num_devices=NCORES)

    ec = nc.dram_tensor("ec", [VC, D], F32, kind="ExternalInput")
    # src points in interleaved order: row r = 128*(j//16) + 16*b + j%16
    src_grid = nc.dram_tensor("src_grid", [NPTS, 3], I32, kind="ExternalInput")
    src_gps = nc.dram_tensor("src_gps", [NPTS, 2], F32, kind="ExternalInput")
    pre_grid = nc.dram_tensor("pre_grid", [NSLOT, 3], I32, kind="ExternalInput")
    next_grid = nc.dram_tensor("next_grid", [NSLOT, 3], I32, kind="ExternalInput")
    # host-precomputed constants (see _host_consts)
    iota512 = nc.dram_tensor("iota512", [P, T], F32, kind="ExternalInput")
    jval = nc.dram_tensor("jval", [P, PPP], F32, kind="ExternalInput")
    c2 = nc.dram_tensor("c2", [P, SPP], F32, kind="ExternalInput")

    src_emb_o = nc.dram_tensor("src_emb_o", [NSLOT, 2 * D], F32, kind="ExternalOutput")
    pre_emb_o = nc.dram_tensor("pre_emb_o", [NSLOT, 2 * D], F32, kind="ExternalOutput")
    next_emb_o = nc.dram_tensor("next_emb_o", [NSLOT, 2 * D], F32, kind="ExternalOutput")
    gps_o = nc.dram_tensor("gps_o", [NSLOT, 3], F32, kind="ExternalOutput")

    # per-point payload rows [x, y, gps_x, gps_y, t, has, 0, 0]; row NPTS = zeros
    payload = nc.dram_tensor("payload", [NPTS + 1, 8], F32)
    # fold bounce [128, 512] and per-slot winner j+1 (0 = empty)
    scr128 = nc.dram_tensor("scr128", [P * T], F32)
    bestj = nc.dram_tensor("bestj", [NSLOT], F32)

    with tile.TileContext(nc) as tc, ExitStack() as ctx:
        pool = ctx.enter_context(tc.tile_pool(name="small", bufs=1))
        epool = ctx.enter_context(tc.tile_pool(name="emb", bufs=16))
        spool = ctx.enter_context(tc.tile_pool(name="stage", bufs=8))

        # ---- stage A: payload table build
        grid_t = pool.tile([P, PPP, 3], I32)
        nc.sync.dma_start(out=grid_t[:], in_=src_grid[:].rearrange("(p c) v -> p c v", p=P))
        gps_t = pool.tile([P, PPP, 2], F32)
        nc.sync.dma_start(out=gps_t[:], in_=src_gps[:].rearrange("(p c) v -> p c v", p=P))

        pay = pool.tile([P, PPP, 8], F32)
        nc.vector.tensor_copy(out=pay[:, :, 0], in_=grid_t[:, :, 0])
        nc.vector.tensor_copy(out=pay[:, :, 1], in_=grid_t[:, :, 1])
        nc.vector.tensor_copy(out=pay[:, :, 2], in_=gps_t[:, :, 0])
        nc.vector.tensor_copy(out=pay[:, :, 3], in_=gps_t[:, :, 1])
        nc.vector.tensor_copy(out=pay[:, :, 4], in_=grid_t[:, :, 2])
        nc.vector.memset(pay[:, :, 5], 1.0)
        nc.vector.memset(pay[:, :, 6:8], 0.0)
        nc.sync.dma_start(out=payload[0:NPTS].rearrange("(p c) v -> p c v", p=P), in_=pay[:])

        zrow = pool.tile([P, 8], F32)
        nc.vector.memset(zrow[:], 0.0)
        nc.sync.dma_start(out=payload[NPTS:NPTS + 1, :], in_=zrow[0:1, :])

        # ---- stage B: winner j+1 per slot via compare/max + folds
        iota_t = pool.tile([P, T], F32)
        nc.sync.dma_start(out=iota_t[:], in_=iota512[:])
        jval_t = pool.tile([P, PPP], F32)
        nc.sync.dma_start(out=jval_t[:], in_=jval[:])

        t_f = pool.tile([P, PPP], F32)
        nc.vector.tensor_copy(out=t_f[:], in_=grid_t[:, :, 2])

        best = pool.tile([P, T], F32)
        nc.vector.memset(best[:], 0.0)
        for c in range(PPP):
            m = pool.tile([P, T], F32, tag="m")
            nc.vector.tensor_scalar(
                out=m[:], in0=iota_t[:], scalar1=t_f[:, c:c + 1],
                scalar2=jval_t[:, c:c + 1],
                op0=mybir.AluOpType.is_equal, op1=MULT)
            nc.vector.tensor_tensor(out=best[:], in0=best[:], in1=m[:], op=MAX)

        # cross-partition max per trajectory (b = p%8): bounce through DRAM,
        # reload with the 16 partials of each trajectory on the free axis.
        nc.sync.dma_start(out=scr128[:].rearrange("(p t) -> p t", p=P), in_=best[:])
        rb = pool.tile([8, 16, T], F32)
        nc.sync.dma_start(out=rb[:], in_=scr128[:].rearrange("(k b t) -> b k t", k=16, b=8))
        g1 = pool.tile([8, 8, T], F32)
        nc.vector.tensor_tensor(out=g1[:], in0=rb[:, 0:8, :], in1=rb[:, 8:16, :], op=MAX)
        g2 = pool.tile([8, 4, T], F32)
        nc.vector.tensor_tensor(out=g2[:], in0=g1[:, 0:4, :], in1=g1[:, 4:8, :], op=MAX)
        g3 = pool.tile([8, 2, T], F32)
        nc.vector.tensor_tensor(out=g3[:], in0=g2[:, 0:2, :], in1=g2[:, 2:4, :], op=MAX)
        m2 = pool.tile([8, T], F32)
        nc.vector.tensor_tensor(out=m2[:].rearrange("p (o t) -> p o t", o=1),
                                in0=g3[:, 0:1, :], in1=g3[:, 1:2, :], op=MAX)
        nc.sync.dma_start(out=bestj[:].rearrange("(b t) -> b t", b=8), in_=m2[:])

        # ---- stage C: winner payload-row index per slot, gather payloads
        bj = pool.tile([P, SPP], F32)
        nc.sync.dma_start(out=bj[:], in_=bestj[:].rearrange("(p n) -> p n", p=P))
        c2_t = pool.tile([P, SPP], F32)
        nc.sync.dma_start(out=c2_t[:], in_=c2[:])

        # payload row r = 128*(j//16) + 16*b + j%16 = 112*(j//16) + j + 16*b
        s1 = pool.tile([P, SPP], F32)
        nc.vector.tensor_scalar(out=s1[:], in0=bj[:], scalar1=1.0, scalar2=None, op0=SUB)
        j_i = pool.tile([P, SPP], I32)
        nc.vector.tensor_copy(out=j_i[:], in_=s1[:])
        jd_i = pool.tile([P, SPP], I32)
        nc.vector.tensor_scalar(out=jd_i[:], in0=j_i[:], scalar1=4, scalar2=None,
                                op0=mybir.AluOpType.arith_shift_right)
        jd_f = pool.tile([P, SPP], F32)
        nc.vector.tensor_copy(out=jd_f[:], in_=jd_i[:])
        t1 = pool.tile([P, SPP], F32)
        nc.vector.tensor_scalar(out=t1[:], in0=jd_f[:], scalar1=112.0, scalar2=None, op0=MULT)
        nc.vector.tensor_tensor(out=t1[:], in0=t1[:], in1=s1[:], op=ADD)
        nc.vector.tensor_tensor(out=t1[:], in0=t1[:], in1=c2_t[:], op=ADD)
        mask = pool.tile([P, SPP], F32)
        nc.vector.tensor_scalar(out=mask[:], in0=bj[:], scalar1=0.0, scalar2=None,
                                op0=mybir.AluOpType.is_gt)
        nc.vector.tensor_scalar(out=t1[:], in0=t1[:], scalar1=float(NPTS), scalar2=None, op0=SUB)
        nc.vector.tensor_tensor(out=t1[:], in0=t1[:], in1=mask[:], op=MULT)
        nc.vector.tensor_scalar(out=t1[:], in0=t1[:], scalar1=float(NPTS), scalar2=None, op0=ADD)
        pidx = pool.tile([P, SPP], I32)
        nc.vector.tensor_copy(out=pidx[:], in_=t1[:])

        # HW indirect DMA only supports [128,1] offsets + contiguous dest:
        # gather payload one slot-column at a time into staging, copy into pg.
        pg = pool.tile([P, SPP, 8], F32)
        for n in range(SPP):
            oc = spool.tile([P, 1], I32, tag="oc")
            nc.vector.tensor_copy(out=oc[:], in_=pidx[:, n:n + 1])
            nc.gpsimd.indirect_dma_start(
                out=pg[:, n, :],
                out_offset=None,
                in_=payload[:],
                in_offset=bass.IndirectOffsetOnAxis(ap=oc[:, :1], axis=0),
            )

        # ---- stage D: new_gps output
        ng = pool.tile([P, SPP, 3], F32)
        nc.vector.tensor_copy(out=ng[:, :, 0], in_=pg[:, :, 2])
        nc.vector.tensor_copy(out=ng[:, :, 1], in_=pg[:, :, 3])
        nc.vector.tensor_copy(out=ng[:, :, 2], in_=pg[:, :, 4])
        nc.sync.dma_start(out=gps_o[:].rearrange("(p n) v -> p n v", p=P), in_=ng[:])

        # ---- stage E: x / y+513 gather offset tiles [128, 32] per tensor
        def make_offsets(name, x_ap, y_ap, cast):
            xi = pool.tile([P, SPP], I32, tag=name + "x")
            yi = pool.tile([P, SPP], I32, tag=name + "y")
            if cast:
                nc.vector.tensor_copy(out=xi[:], in_=x_ap)
                yf = pool.tile([P, SPP], F32, tag=name + "yf")
                nc.vector.tensor_scalar(out=yf[:], in0=y_ap, scalar1=513.0, scalar2=None,
                                        op0=ADD)
                nc.vector.tensor_copy(out=yi[:], in_=yf[:])
            else:
                nc.vector.tensor_copy(out=xi[:], in_=x_ap)
                nc.vector.tensor_scalar(out=yi[:], in0=y_ap, scalar1=513, scalar2=None,
                                        op0=ADD)
            return xi, yi

        s_off = make_offsets("s", pg[:, :, 0], pg[:, :, 1], cast=True)

        pgrid_t = pool.tile([P, SPP, 3], I32)
        nc.sync.dma_start(out=pgrid_t[:], in_=pre_grid[:].rearrange("(p n) v -> p n v", p=P))
        p_off = make_offsets("p", pgrid_t[:, :, 0], pgrid_t[:, :, 1], cast=False)

        ngrid_t = pool.tile([P, SPP, 3], I32)
        nc.sync.dma_start(out=ngrid_t[:], in_=next_grid[:].rearrange("(p n) v -> p n v", p=P))
        n_off = make_offsets("n", ngrid_t[:, :, 0], ngrid_t[:, :, 1], cast=False)

        # ---- stage F: embedding gathers + stores, one 128-row column at a time
        for (xi, yi), out_t in ((p_off, pre_emb_o), (n_off, next_emb_o),
                                (s_off, src_emb_o)):
            out_v = out_t[:].rearrange("(p n) d -> p n d", p=P)
            for n in range(SPP):
                et = epool.tile([P, 2 * D], F32, tag="et")
                for h, idx_t in ((0, xi), (1, yi)):
                    oc = spool.tile([P, 1], I32, tag="eoc")
                    nc.vector.tensor_copy(out=oc[:], in_=idx_t[:, n:n + 1])
                    nc.gpsimd.indirect_dma_start(
                        out=et[:, h * D:(h + 1) * D],
                        out_offset=None,
                        in_=ec[:],
                        in_offset=bass.IndirectOffsetOnAxis(ap=oc[:, :1], axis=0),
                    )
                eng = nc.sync if n % 2 == 0 else nc.scalar
                eng.dma_start(out=out_v[:, n, :], in_=et[:])

    nc.compile()
    return nc


def _host_consts():
    p = np.arange(P)[:, None]
    iota512 = np.broadcast_to(np.arange(T, dtype=np.float32)[None, :], (P, T)).copy()
    # point in tile slot (p, c): trajectory b = p%8, j = 16*(p//8) + c
    jval = (16 * (p // 8) + np.arange(PPP)[None, :] + 1).astype(np.float32)
    c2 = np.broadcast_to((16 * (p // 16)).astype(np.float32), (P, SPP)).copy()
    return iota512, jval, c2


# permutation: interleaved row r = 128*g + 16*b + c holds point q = 256*b + 16*g + c
_R = np.arange(NPTS)
_QIDX = 256 * ((_R // 16) % 8) + 16 * (_R // 128) + (_R % 16)


def make_in_maps(inputs):
    src_grid_seqs = np.ascontiguousarray(inputs["src_grid_seqs"], dtype=np.int32)
    src_gps_seqs = np.ascontiguousarray(inputs["src_gps_seqs"], dtype=np.float32)
    pre_grids = np.ascontiguousarray(inputs["pre_grids"], dtype=np.int32)
    next_grids = np.ascontiguousarray(inputs["next_grids"], dtype=np.int32)
    E1 = np.ascontiguousarray(inputs["E1"], dtype=np.float32)
    E2 = np.ascontiguousarray(inputs["E2"], dtype=np.float32)

    ec = np.ascontiguousarray(np.concatenate([E1, E2], axis=0))
    iota512, jval, c2 = _host_consts()

    in_maps = []
    for c in range(NCORES):
        in_maps.append({
            "ec": ec,
            "src_grid": np.ascontiguousarray(
                src_grid_seqs[c * BC:(c + 1) * BC].reshape(NPTS, 3)[_QIDX]),
            "src_gps": np.ascontiguousarray(
                src_gps_seqs[c * BC:(c + 1) * BC].reshape(NPTS, 2)[_QIDX]),
            "pre_grid": np.ascontiguousarray(
                pre_grids[c * TC:(c + 1) * TC].reshape(NSLOT, 3)),
            "next_grid": np.ascontiguousarray(
                next_grids[c * TC:(c + 1) * TC].reshape(NSLOT, 3)),
            "iota512": iota512, "jval": jval, "c2": c2,
        })
    return in_maps


def run(inputs, trace=False):
    nc = build_nc()
    in_maps = make_in_maps(inputs)
    res = run_bass_kernel_spmd(nc, in_maps, list(range(NCORES)), trace=trace)
    rs = res.results

    src_emb = np.concatenate(
        [rs[c]["src_emb_o"].reshape(BC, T, 2 * D) for c in range(NCORES)], axis=0)
    pre_emb = np.concatenate(
        [rs[c]["pre_emb_o"].reshape(TC, B, 2 * D) for c in range(NCORES)], axis=0)
    next_emb = np.concatenate(
        [rs[c]["next_emb_o"].reshape(TC, B, 2 * D) for c in range(NCORES)], axis=0)
    new_gps = np.concatenate(
        [rs[c]["gps_o"].reshape(BC, T, 3) for c in range(NCORES)], axis=0)
    return (src_emb, pre_emb, next_emb, new_gps), res


def kernel(**inputs):
    return run(inputs, trace=False)[0]


# revision 19
# speedup vs baseline: 1.1485x; 1.1485x over previous
"""Trainium2 Bass kernel for DTPTrajRec-style scatter + embedding gathers.

Computes, for full inputs:
  - new_grid/new_gps scatter (last-write-wins per (trajectory, time slot))
  - src_emb = E1[new_grid[...,0]] ++ E2[new_grid[...,1]]   [B, T, 2D]
  - pre_emb/next_emb = E1[g[...,0]] ++ E2[g[...,1]]        [T, B, 2D]
Sharded over 8 NeuronCores: batch-parallel for the src/scatter path,
T-parallel for pre/next. Embedding tables are replicated (concatenated
into one [1026, 512] table so one indirect-DMA gather fetches either).

Last-write-wins winner (max source index j per (trajectory, slot)) is
computed on the vector engine: 16 fused is_equal*(j+1) compares against
a slot-iota, max-accumulated, then partition-folded. Source points are
interleaved on the host (partition p holds points of trajectory p%8) so
partition folds stay within one trajectory.
"""
import sys

for _p in ("/opt/trn_rl_repo", "/root/.axon_site/_ro/trn_rl_repo"):
    if _p not in sys.path:
        sys.path.insert(0, _p)

from contextlib import ExitStack

import numpy as np

import concourse.bass as bass
import concourse.tile as tile
from concourse import bacc, mybir
from concourse.bass_utils import run_bass_kernel_spmd

P = 128
NCORES = 8
B, L, T = 64, 256, 512
D = 512
VC = 2 * 513            # combined table rows (E1 then E2)
BC = B // NCORES        # 8 trajectories per core
TC = T // NCORES        # 64 pre/next time rows per core
NPTS = BC * L           # 2048 source points per core
NSLOT = BC * T          # 4096 output rows per core (= TC * B as well)
PPP = NPTS // P         # 16 points per partition
SPP = NSLOT // P        # 32 output rows per partition
KG = 8                  # embedding rows gathered per indirect DMA per partition
NG = SPP // KG          # 4 gather groups per output tensor

F32 = mybir.dt.float32
I32 = mybir.dt.int32
ADD = mybir.AluOpType.add
SUB = mybir.AluOpType.subtract
MULT = mybir.AluOpType.mult
MAX = mybir.AluOpType.max


def build_nc():
    nc = bacc.Bacc("TRN2", target_bir_lowering=False, debug=False,
                   num_devices=NCORES)

    ec = nc.dram_tensor("ec", [VC, D], F32, kind="ExternalInput")
    # src points in interleaved order: row r = 128*(j//16) + 16*b + j%16
    src_grid = nc.dram_tensor("src_grid", [NPTS, 3], I32, kind="ExternalInput")
    src_gps = nc.dram_tensor("src_gps", [NPTS, 2], F32, kind="ExternalInput")
    pre_grid = nc.dram_tensor("pre_grid", [NSLOT, 3], I32, kind="ExternalInput")
    next_grid = nc.dram_tensor("next_grid", [NSLOT, 3], I32, kind="ExternalInput")
    # host-precomputed constants (see _host_consts)
    iota512 = nc.dram_tensor("iota512", [P, T], F32, kind="ExternalInput")
    jval = nc.dram_tensor("jval", [P, PPP], F32, kind="ExternalInput")
    c2 = nc.dram_tensor("c2", [P, SPP], F32, kind="ExternalInput")

    src_emb_o = nc.dram_tensor("src_emb_o", [NSLOT, 2 * D], F32, kind="ExternalOutput")
    pre_emb_o = nc.dram_tensor("pre_emb_o", [NSLOT, 2 * D], F32, kind="ExternalOutput")
    next_emb_o = nc.dram_tensor("next_emb_o", [NSLOT, 2 * D], F32, kind="ExternalOutput")
    gps_o = nc.dram_tensor("gps_o", [NSLOT, 3], F32, kind="ExternalOutput")

    # per-point payload rows [x, y, gps_x, gps_y, t, has, 0, 0]; row NPTS = zeros
    payload = nc.dram_tensor("payload", [NPTS + 1, 8], F32)
    # fold bounce [128, 512] and per-slot winner j+1 (0 = empty)
    scr128 = nc.dram_tensor("scr128", [P * T], F32)
    bestj = nc.dram_tensor("bestj", [NSLOT], F32)

    with tile.TileContext(nc) as tc, ExitStack() as ctx:
        pool = ctx.enter_context(tc.tile_pool(name="small", bufs=1))
        epool = ctx.enter_context(tc.tile_pool(name="emb", bufs=12))
        spool = ctx.enter_context(tc.tile_pool(name="stage", bufs=8))

        # ---- stage A: payload table build
        grid_t = pool.tile([P, PPP, 3], I32)
        nc.sync.dma_start(out=grid_t[:], in_=src_grid[:].rearrange("(p c) v -> p c v", p=P))
        gps_t = pool.tile([P, PPP, 2], F32)
        nc.sync.dma_start(out=gps_t[:], in_=src_gps[:].rearrange("(p c) v -> p c v", p=P))

        pay = pool.tile([P, PPP, 8], F32)
        nc.vector.tensor_copy(out=pay[:, :, 0], in_=grid_t[:, :, 0])
        nc.vector.tensor_copy(out=pay[:, :, 1], in_=grid_t[:, :, 1])
        nc.vector.tensor_copy(out=pay[:, :, 2], in_=gps_t[:, :, 0])
        nc.vector.tensor_copy(out=pay[:, :, 3], in_=gps_t[:, :, 1])
        nc.vector.tensor_copy(out=pay[:, :, 4], in_=grid_t[:, :, 2])
        nc.vector.memset(pay[:, :, 5], 1.0)
        nc.vector.memset(pay[:, :, 6:8], 0.0)
        nc.sync.dma_start(out=payload[0:NPTS].rearrange("(p c) v -> p c v", p=P), in_=pay[:])

        zrow = pool.tile([P, 8], F32)
        nc.vector.memset(zrow[:], 0.0)
        nc.sync.dma_start(out=payload[NPTS:NPTS + 1, :], in_=zrow[0:1, :])

        # ---- stage B: winner j+1 per slot via compare/max + folds
        iota_t = pool.tile([P, T], F32)
        nc.sync.dma_start(out=iota_t[:], in_=iota512[:])
        jval_t = pool.tile([P, PPP], F32)
        nc.sync.dma_start(out=jval_t[:], in_=jval[:])

        t_f = pool.tile([P, PPP], F32)
        nc.vector.tensor_copy(out=t_f[:], in_=grid_t[:, :, 2])

        best = pool.tile([P, T], F32)
        nc.vector.memset(best[:], 0.0)
        for c in range(PPP):
            m = pool.tile([P, T], F32, tag="m")
            nc.vector.tensor_scalar(
                out=m[:], in0=iota_t[:], scalar1=t_f[:, c:c + 1],
                scalar2=jval_t[:, c:c + 1],
                op0=mybir.AluOpType.is_equal, op1=MULT)
            nc.vector.tensor_tensor(out=best[:], in0=best[:], in1=m[:], op=MAX)

        # cross-partition max per trajectory (b = p%8): bounce through DRAM,
        # reload with the 16 partials of each trajectory on the free axis.
        nc.sync.dma_start(out=scr128[:].rearrange("(p t) -> p t", p=P), in_=best[:])
        rb = pool.tile([8, 16, T], F32)
        nc.sync.dma_start(out=rb[:], in_=scr128[:].rearrange("(k b t) -> b k t", k=16, b=8))
        g1 = pool.tile([8, 8, T], F32)
        nc.vector.tensor_tensor(out=g1[:], in0=rb[:, 0:8, :], in1=rb[:, 8:16, :], op=MAX)
        g2 = pool.tile([8, 4, T], F32)
        nc.vector.tensor_tensor(out=g2[:], in0=g1[:, 0:4, :], in1=g1[:, 4:8, :], op=MAX)
        g3 = pool.tile([8, 2, T], F32)
        nc.vector.tensor_tensor(out=g3[:], in0=g2[:, 0:2, :], in1=g2[:, 2:4, :], op=MAX)
        m2 = pool.tile([8, T], F32)
        nc.vector.tensor_tensor(out=m2[:].rearrange("p (o t) -> p o t", o=1),
                                in0=g3[:, 0:1, :], in1=g3[:, 1:2, :], op=MAX)
        nc.sync.dma_start(out=bestj[:].rearrange("(b t) -> b t", b=8), in_=m2[:])

        # ---- stage C: winner payload-row index per slot, gather payloads
        bj = pool.tile([P, SPP], F32)
        nc.sync.dma_start(out=bj[:], in_=bestj[:].rearrange("(p n) -> p n", p=P))
        c2_t = pool.tile([P, SPP], F32)
        nc.sync.dma_start(out=c2_t[:], in_=c2[:])

        # payload row r = 128*(j//16) + 16*b + j%16 = 112*(j//16) + j + 16*b
        s1 = pool.tile([P, SPP], F32)
        nc.vector.tensor_scalar(out=s1[:], in0=bj[:], scalar1=1.0, scalar2=None, op0=SUB)
        j_i = pool.tile([P, SPP], I32)
        nc.vector.tensor_copy(out=j_i[:], in_=s1[:])
        jd_i = pool.tile([P, SPP], I32)
        nc.vector.tensor_scalar(out=jd_i[:], in0=j_i[:], scalar1=4, scalar2=None,
                                op0=mybir.AluOpType.arith_shift_right)
        jd_f = pool.tile([P, SPP], F32)
        nc.vector.tensor_copy(out=jd_f[:], in_=jd_i[:])
        t1 = pool.tile([P, SPP], F32)
        nc.vector.tensor_scalar(out=t1[:], in0=jd_f[:], scalar1=112.0, scalar2=None, op0=MULT)
        nc.vector.tensor_tensor(out=t1[:], in0=t1[:], in1=s1[:], op=ADD)
        nc.vector.tensor_tensor(out=t1[:], in0=t1[:], in1=c2_t[:], op=ADD)
        mask = pool.tile([P, SPP], F32)
        nc.vector.tensor_scalar(out=mask[:], in0=bj[:], scalar1=0.0, scalar2=None,
                                op0=mybir.AluOpType.is_gt)
        nc.vector.tensor_scalar(out=t1[:], in0=t1[:], scalar1=float(NPTS), scalar2=None, op0=SUB)
        nc.vector.tensor_tensor(out=t1[:], in0=t1[:], in1=mask[:], op=MULT)
        nc.vector.tensor_scalar(out=t1[:], in0=t1[:], scalar1=float(NPTS), scalar2=None, op0=ADD)
        pidx = pool.tile([P, SPP], I32)
        nc.vector.tensor_copy(out=pidx[:], in_=t1[:])

        pg = pool.tile([P, SPP, 8], F32)

        def emit_payload_gather(n):
            oc = spool.tile([P, 1], I32, tag="oc")
            nc.vector.tensor_copy(out=oc[:], in_=pidx[:, n:n + 1])
            nc.gpsimd.indirect_dma_start(
                out=pg[:, n, :],
                out_offset=None,
                in_=payload[:],
                in_offset=bass.IndirectOffsetOnAxis(ap=oc[:, :1], axis=0),
            )

        # ---- stage E: x / y+513 gather offset tiles [128, 32] per tensor
        def make_offsets(name, x_ap, y_ap, cast):
            xi = pool.tile([P, SPP], I32, tag=name + "x")
            yi = pool.tile([P, SPP], I32, tag=name + "y")
            if cast:
                nc.vector.tensor_copy(out=xi[:], in_=x_ap)
                yf = pool.tile([P, SPP], F32, tag=name + "yf")
                nc.vector.tensor_scalar(out=yf[:], in0=y_ap, scalar1=513.0, scalar2=None,
                                        op0=ADD)
                nc.vector.tensor_copy(out=yi[:], in_=yf[:])
            else:
                nc.vector.tensor_copy(out=xi[:], in_=x_ap)
                nc.vector.tensor_scalar(out=yi[:], in0=y_ap, scalar1=513, scalar2=None,
                                        op0=ADD)
            return xi, yi

        pgrid_t = pool.tile([P, SPP, 3], I32)
        nc.sync.dma_start(out=pgrid_t[:], in_=pre_grid[:].rearrange("(p n) v -> p n v", p=P))
        p_off = make_offsets("p", pgrid_t[:, :, 0], pgrid_t[:, :, 1], cast=False)

        ngrid_t = pool.tile([P, SPP, 3], I32)
        nc.sync.dma_start(out=ngrid_t[:], in_=next_grid[:].rearrange("(p n) v -> p n v", p=P))
        n_off = make_offsets("n", ngrid_t[:, :, 0], ngrid_t[:, :, 1], cast=False)

        # ---- stage F: embedding gathers + stores, one 128-row column at a time.
        # Payload gathers are interleaved into the next_emb stream so the Pool
        # engine never runs a payload-only block that starves the SDMA engines.
        def emb_column(xi, yi, out_v, n):
            et = epool.tile([P, 2 * D], F32, tag="et")
            for h, idx_t in ((0, xi), (1, yi)):
                oc = spool.tile([P, 1], I32, tag="eoc")
                nc.vector.tensor_copy(out=oc[:], in_=idx_t[:, n:n + 1])
                nc.gpsimd.indirect_dma_start(
                    out=et[:, h * D:(h + 1) * D],
                    out_offset=None,
                    in_=ec[:],
                    in_offset=bass.IndirectOffsetOnAxis(ap=oc[:, :1], axis=0),
                )
            eng = nc.sync if n % 2 == 0 else nc.scalar
            eng.dma_start(out=out_v[:, n, :], in_=et[:])

        pre_v = pre_emb_o[:].rearrange("(p n) d -> p n d", p=P)
        next_v = next_emb_o[:].rearrange("(p n) d -> p n d", p=P)
        src_v = src_emb_o[:].rearrange("(p n) d -> p n d", p=P)

        for n in range(SPP):
            emb_column(*p_off, pre_v, n)
        for n in range(SPP):
            emb_column(*n_off, next_v, n)
            emit_payload_gather(n)

        # ---- stage D: new_gps output (needs pg complete)
        ng = pool.tile([P, SPP, 3], F32)
        nc.vector.tensor_copy(out=ng[:, :, 0], in_=pg[:, :, 2])
        nc.vector.tensor_copy(out=ng[:, :, 1], in_=pg[:, :, 3])
        nc.vector.tensor_copy(out=ng[:, :, 2], in_=pg[:, :, 4])
        nc.sync.dma_start(out=gps_o[:].rearrange("(p n) v -> p n v", p=P), in_=ng[:])

        s_off = make_offsets("s", pg[:, :, 0], pg[:, :, 1], cast=True)
        for n in range(SPP):
            emb_column(*s_off, src_v, n)

    nc.compile()
    return nc


def _host_consts():
    p = np.arange(P)[:, None]
    iota512 = np.broadcast_to(np.arange(T, dtype=np.float32)[None, :], (P, T)).copy()
    # point in tile slot (p, c): trajectory b = p%8, j = 16*(p//8) + c
    jval = (16 * (p // 8) + np.arange(PPP)[None, :] + 1).astype(np.float32)
    c2 = np.broadcast_to((16 * (p // 16)).astype(np.float32), (P, SPP)).copy()
    return iota512, jval, c2


# permutation: interleaved row r = 128*g + 16*b + c holds point q = 256*b + 16*g + c
_R = np.arange(NPTS)
_QIDX = 256 * ((_R // 16) % 8) + 16 * (_R // 128) + (_R % 16)


def make_in_maps(inputs):
    src_grid_seqs = np.ascontiguousarray(inputs["src_grid_seqs"], dtype=np.int32)
    src_gps_seqs = np.ascontiguousarray(inputs["src_gps_seqs"], dtype=np.float32)
    pre_grids = np.ascontiguousarray(inputs["pre_grids"], dtype=np.int32)
    next_grids = np.ascontiguousarray(inputs["next_grids"], dtype=np.int32)
    E1 = np.ascontiguousarray(inputs["E1"], dtype=np.float32)
    E2 = np.ascontiguousarray(inputs["E2"], dtype=np.float32)

    ec = np.ascontiguousarray(np.concatenate([E1, E2], axis=0))
    iota512, jval, c2 = _host_consts()

    in_maps = []
    for c in range(NCORES):
        in_maps.append({
            "ec": ec,
            "src_grid": np.ascontiguousarray(
                src_grid_seqs[c * BC:(c + 1) * BC].reshape(NPTS, 3)[_QIDX]),
            "src_gps": np.ascontiguousarray(
                src_gps_seqs[c * BC:(c + 1) * BC].reshape(NPTS, 2)[_QIDX]),
            "pre_grid": np.ascontiguousarray(
                pre_grids[c * TC:(c + 1) * TC].reshape(NSLOT, 3)),
            "next_grid": np.ascontiguousarray(
                next_grids[c * TC:(c + 1) * TC].reshape(NSLOT, 3)),
            "iota512": iota512, "jval": jval, "c2": c2,
        })
    return in_maps


def run(inputs, trace=False):
    nc = build_nc()
    in_maps = make_in_maps(inputs)
    res = run_bass_kernel_spmd(nc, in_maps, list(range(NCORES)), trace=trace)
    rs = res.results

    src_emb = np.concatenate(
        [rs[c]["src_emb_o"].reshape(BC, T, 2 * D) for c in range(NCORES)], axis=0)
    pre_emb = np.concatenate(
        [rs[c]["pre_emb_o"].reshape(TC, B, 2 * D) for c in range(NCORES)], axis=0)
    next_emb = np.concatenate(
        [rs[c]["next_emb_o"].reshape(TC, B, 2 * D) for c in range(NCORES)], axis=0)
    new_gps = np.concatenate(
        [rs[c]["gps_o"].reshape(BC, T, 3) for c in range(NCORES)], axis=0)
    return (src_emb, pre_emb, next_emb, new_gps), res


def kernel(**inputs):
    return run(inputs, trace=False)[0]
